# revision 58
# baseline (speedup 1.0000x reference)
"""Trainium2 Bass kernel for LocalSparseAttention.

Problem (hardcoded): B=2, S=2048, D=1024, H=16, HD=64, WINDOW=128 (band
|i-j| <= 64), fp32 I/O.

Sharding: 8 cores = 2 batches x 4 head-groups (4 heads each). Each core:
  - qk projection into transposed layout [512, 2048] (head-pair packed)
  - v projection: 16 column-aligned seq chunks on the PE; the 15
    64-shifted chunks the banded attention needs are assembled by
    SBUF->SBUF partition-shift DMAs on the idle mid-kernel queue
  - banded attention, per-pair (256-query) scores with the two heads'
    K=64 matmuls interleaved (they co-execute in disjoint PE row-groups),
    exp on ACT into a combined per-pair tile, band mask as one DVE
    multiply (mask variants selected via broadcast APs from a compact
    [128,3,2,128] table), then per-block AV + normalization: a
    ones-matmul reads the exp tile directly and lands the softmax
    denominator PRE-BroADCAST in PSUM (denbc), so the reciprocal runs on
    DVE concurrently with the AV matmuls
  - output projection staged in 2-st slots -> one 512KB DMA per slot
Host: fp16 casts/transposes in, sum of 4 partials per batch + fused bias
(b_out + b_v @ w_out) out.

Schedule notes:
  - input DMAs carved (kt, ns)-wise, useful-first, first feed split
    across the Sync+ACT HWDGE queues; ~14 warmup matmuls on zeroed SBUF
    bridge the HAM clock-gate to the first feed's landing (the per-core
    DMA start jitters by ~1-2us; scoring is max-core)
  - qk0's (m1, m3) bias ops are deferred past the first scores unit:
    dependency tracking is tile-granular, so emitting them earlier would
    stall the unit on bias ops it does not read
  - every disjoint PSUM region's first matmul carries start=True: the
    hardware's has_written clear does NOT cover partitions the starting
    matmul doesn't write (stale bits => silent accumulation corruption)
  - emission order == per-engine execution order: each scores unit is
    followed by independent filler work (projection chunks / outproj
    tiles) sized to cover the exp+mask latency

All matmuls run in fp16 (1 cycle/row on PE warm at 2.4GHz) with fp32
PSUM accumulation; softmax exp input stays fp32. Interior score pairs
merge the shared key chunk into one 256-col matmul (3 matmuls/head
instead of 4). Outproj PSUM->SBUF copies alternate ACT/DVE to
unserialize the tail; the last 4 outproj units draw PSUM from the
then-idle scores pool. qk bias ops split DVE/ACT per chunk. Measured
96.5-97.8us max-core / ~95.8-96.4us mean over repeat runs (prev
session 103.2us; run-to-run max-core jitter ~1us), rel err 7.7e-4. fp8 (DoubleRow) for the
softmax-denominator matmul was tried and rejected: attention is peaked
enough that e4m3 quantization of exp does not average out
(rel err 1.8e-2, at the gate).
"""
import sys

if "/opt/trn_rl_repo" not in sys.path:
    sys.path.insert(0, "/opt/trn_rl_repo")

import numpy as np

import concourse.bass as bass
import concourse.mybir as mybir
import concourse.tile as tile
from concourse import bacc
from concourse.bass_utils import run_bass_kernel_spmd

B, S, D, H, HD = 2, 2048, 1024, 16, 64
SCALE = HD**-0.5
C_SUB = 4.0  # subtracted from all scores via the mask; cancels in softmax
MASK_NEG = -30000.0

F16 = mybir.dt.float16
F32 = mybir.dt.float32
F32R = mybir.dt.float32r
WARMUP_MM = 20

# 19 key/value chunk offsets: 15 shifted (128c+64) + aligned 0,128,1792,1920
OFFS = [128 * c + 64 for c in range(15)] + [0, 128, 1792, 1920]
# v_sb slot per chunk id: shifted c -> slot c; aligned 128a -> slot 16+a
VSLOT = {c: c for c in range(15)}
VSLOT.update({15: 16, 16: 17, 17: 30, 18: 31})


def _chunk_pair(i):
    if i == 0:
        return 15, 16
    if i == 15:
        return 17, 18
    return i - 1, i


def _build_pair_masks():
    # variant 0: (first, interior) — c4=0 pair 0
    # variant 1: (interior, interior)
    # variant 2: (interior, last)  — c4=3 pair 1
    m = _build_masks()  # [128, 3(first/int/last), 2(half), 128]
    mp = np.zeros((128, 3, 2, 2, 128), np.float16)
    mp[:, 0, 0] = m[:, 0]
    mp[:, 0, 1] = m[:, 1]
    mp[:, 1, 0] = m[:, 1]
    mp[:, 1, 1] = m[:, 1]
    mp[:, 2, 0] = m[:, 1]
    mp[:, 2, 1] = m[:, 2]
    return mp


def _build_masks():
    kp = np.arange(128)[:, None]
    p = np.arange(128)[None, :]
    masks = np.zeros((128, 3, 2, 128), np.float16)
    for v, shift in enumerate([0, 64, 128]):
        for half in (0, 1):
            w = 128 * half + kp
            valid = np.abs(p + shift - w) <= 64
            masks[:, v, half, :] = valid.astype(np.float16)
    return masks


def _build_program():
    nc = bacc.Bacc("TRN2", debug=False, num_devices=8)

    xT_d = nc.dram_tensor("xT", [D, S], F16, kind="ExternalInput").ap()
    # wqk/wv arrive host-rearranged to [kp, ko, n] so their DMA rows are
    # contiguous 2KB/4KB lines instead of the 1KB/512B strided rows a
    # device-side rearrange view would produce (input stream is the
    # binding constraint for the first ~25us)
    wqk_d = nc.dram_tensor("wqk", [128, 8, 512], F16,
                           kind="ExternalInput").ap()
    wv_d = nc.dram_tensor("wv", [128, 8, 256], F16,
                          kind="ExternalInput").ap()
    wout_d = nc.dram_tensor("wout", [256, D], F16, kind="ExternalInput").ap()
    bqk_d = nc.dram_tensor("bqk", [128, 4], F32, kind="ExternalInput").ap()
    masks_d = nc.dram_tensor("masks", [128, 3, 2, 128], F16,
                             kind="ExternalInput").ap()
    out_d = nc.dram_tensor("out", [S, D], F16, kind="ExternalOutput").ap()

    with tile.TileContext(nc) as tc:
        with (
            tc.tile_pool(name="const", bufs=1) as cpool,
            tc.tile_pool(name="work", bufs=3) as wpool,
            tc.tile_pool(name="expp", bufs=6) as epool,
            tc.tile_pool(name="ysb", bufs=5) as ypool,
            tc.tile_pool(name="ps512", bufs=2, space="PSUM") as ps512,
            tc.tile_pool(name="psv", bufs=1, space="PSUM") as psv,
            tc.tile_pool(name="pssc", bufs=2, space="PSUM") as pssc,
            tc.tile_pool(name="psav", bufs=1, space="PSUM") as psav,
        ):
            # ---- persistent SBUF tensors ----
            xT_sb = cpool.tile([128, 8, S], F16, tag="xT")
            wqk_sb = cpool.tile([128, 8, 512], F16, tag="wqk")
            wv_sb = cpool.tile([128, 8, 256], F16, tag="wv")
            wout_sb = cpool.tile([128, 2, D], F16, tag="wout")
            bqk_sb = cpool.tile([128, 4], F32, tag="bqk")
            masks_sb = cpool.tile([128, 1, 3, 2, 128], F16, tag="masks")
            qk_sb = cpool.tile([128, 4, S], F16, tag="qk")
            v_sb = cpool.tile([128, 32, 4, 64], F16, tag="v")
            aoT_sb = cpool.tile([128, 2, S], F16, tag="aoT")
            ones_sb = cpool.tile([128, 64], F16, tag="ones")
            negc_sb = cpool.tile([128, 1], F32, tag="negc")

            xT_r = xT_d.rearrange("(ko kp) s -> kp ko s", kp=128)
            wqk_r = wqk_d
            wv_r = wv_d

            # ---- input DMAs ----
            # First feed split across the two HWDGE queues so the two
            # halves stream concurrently: wqk[0:2] on Sync, its xT mate on
            # the ACT queue (idle early). bqk follows on ACT (needed only
            # at the first qk TS ~19us).
            nc.scalar.dma_start(out=xT_sb[:, 0:2, 0:512],
                                in_=xT_r[:, 0:2, 0:512])
            nc.scalar.dma_start(out=bqk_sb[:], in_=bqk_d)
            # Sync queue, priority order, kt-pairwise so qk0's kt-outer
            # loop can start each kt slice as soon as its pair lands.
            nc.sync.dma_start(out=wqk_sb[:, 0:2], in_=wqk_r[:, 0:2])
            for k0 in (2, 4, 6):
                nc.sync.dma_start(out=wqk_sb[:, k0:k0 + 2],
                                  in_=wqk_r[:, k0:k0 + 2])
                nc.sync.dma_start(out=xT_sb[:, k0:k0 + 2, 0:512],
                                  in_=xT_r[:, k0:k0 + 2, 0:512])
            nc.sync.dma_start(out=wv_sb[:], in_=wv_r[:])
            nc.sync.dma_start(out=xT_sb[:, :, 512:1024],
                              in_=xT_r[:, :, 512:1024])
            nc.sync.dma_start(out=masks_sb[:, 0], in_=masks_d)
            nc.sync.dma_start(out=xT_sb[:, :, 1024:1536],
                              in_=xT_r[:, :, 1024:1536])
            nc.sync.dma_start(out=xT_sb[:, :, 1536:2048],
                              in_=xT_r[:, :, 1536:2048])
            nc.sync.dma_start(
                out=wout_sb[:],
                in_=wout_d.rearrange("(t p) n -> p t n", p=128),
            )

            wsrc = cpool.tile([128, 256], F16, tag="wsrc")
            nc.vector.memset(wsrc[:], 0.0)
            nc.vector.memset(ones_sb[:], 1.0)
            nc.vector.memset(negc_sb[:], -C_SUB)

            # ---- PE warmup: dummy matmuls on zeroed SBUF bridge the HAM
            # clock-gate ramp until the first feed's landing (~8.3us with
            # the split-queue first feed) ----
            wps = psv.tile([128, 256], F32, tag="psv")
            for w in range(WARMUP_MM):
                nc.tensor.matmul(
                    out=wps[:],
                    lhsT=wsrc[:, 0:128],
                    rhs=wsrc[:],
                    start=(w == 0),
                    stop=(w == WARMUP_MM - 1),
                )
            wdst = wpool.tile([128, 256], F16, tag="wdst")
            nc.scalar.copy(out=wdst[:], in_=wps[:])

            # ---- emission helpers ----
            def _qk0_ts(tiles, ms):
                # bias+scale split across DVE (m=0,1) and ACT (m=2,3) so
                # the pair a scores unit needs runs on two engines at once
                for m in ms:
                    scale = SCALE if m < 2 else 1.0
                    if m < 2:
                        nc.vector.tensor_scalar(
                            out=qk_sb[:, m, 0:512],
                            in0=tiles[m][:],
                            scalar1=scale,
                            scalar2=bqk_sb[:, m:m + 1],
                            op0=mybir.AluOpType.mult,
                            op1=mybir.AluOpType.add,
                        )
                    else:
                        nc.scalar.activation(
                            out=qk_sb[:, m, 0:512],
                            in_=tiles[m][:],
                            func=mybir.ActivationFunctionType.Identity,
                            bias=bqk_sb[:, m:m + 1],
                            scale=scale,
                        )

            def emit_qk_ns0_ktouter():
                # first qk chunk, kt-outer so matmuls start as soon as the
                # first (wqk, xT) kt-slices land; 4 psum banks open at
                # once. m-order (0,2,1,3) so the hp=0 pair's PSUM closes
                # first. Only the (m0, m2) bias ops are emitted here — the
                # (m1, m3) ops are deferred past the first scores unit
                # (tile-granular dependency tracking would otherwise make
                # it wait on all four).
                rrp = [(ps512, "ps512"), (pssc, "pssc"), (psav, "psav"),
                       (ps512, "ps512")]
                tiles = [pool.tile([128, 512], F32, tag=tg, name=f"qk0m{m}")
                         for m, (pool, tg) in enumerate(rrp)]
                for kt in range(8):
                    for m in (0, 2, 1, 3):
                        nc.tensor.matmul(
                            out=tiles[m][:],
                            lhsT=wqk_sb[:, kt, m * 128:(m + 1) * 128],
                            rhs=xT_sb[:, kt, 0:512],
                            start=(kt == 0),
                            stop=(kt == 7),
                        )
                _qk0_ts(tiles, (0, 2))
                return tiles

            def emit_qk_ns(ns):
                for m in (0, 2, 1, 3):
                    scale = SCALE if m < 2 else 1.0
                    ps = ps512.tile([128, 512], F32, tag="ps512")
                    for kt in range(8):
                        nc.tensor.matmul(
                            out=ps[:],
                            lhsT=wqk_sb[:, kt, m * 128:(m + 1) * 128],
                            rhs=xT_sb[:, kt, ns * 512:(ns + 1) * 512],
                            start=(kt == 0),
                            stop=(kt == 7),
                        )
                    # bias+scale on DVE for m=0,1 and ACT for m=2,3 so the
                    # (q, k) pair a scores unit reads finishes ~2x sooner
                    if m < 2:
                        nc.vector.tensor_scalar(
                            out=qk_sb[:, m, ns * 512:(ns + 1) * 512],
                            in0=ps[:],
                            scalar1=scale,
                            scalar2=bqk_sb[:, m:m + 1],
                            op0=mybir.AluOpType.mult,
                            op1=mybir.AluOpType.add,
                        )
                    else:
                        nc.scalar.activation(
                            out=qk_sb[:, m, ns * 512:(ns + 1) * 512],
                            in_=ps[:],
                            func=mybir.ActivationFunctionType.Identity,
                            bias=bqk_sb[:, m:m + 1],
                            scale=scale,
                        )

            def emit_v_aligned(a):
                # aligned v chunk a covers seq [128a, 128a+128); the
                # 64-shifted chunks are assembled later by partition-shift
                # DMAs instead of redundant matmuls
                off = 128 * a
                ps = psv.tile([128, 256], F32, tag="psv")
                for kt in range(8):
                    nc.tensor.matmul(
                        out=ps[:],
                        lhsT=xT_sb[:, kt, off:off + 128],
                        rhs=wv_sb[:, kt, :],
                        start=(kt == 0),
                        stop=(kt == 7),
                    )
                nc.scalar.copy(
                    out=v_sb[:, 16 + a],
                    in_=ps[:].rearrange("p (h d) -> p h d", h=4),
                )

            def emit_vshift(g0, g1):
                # shifted chunk c = aligned(c) rows 64:128 ++ aligned(c+1)
                # rows 0:64; two strided SBUF->SBUF DMAs build chunks
                # [g0, g1) from aligned slots on the idle mid-kernel queue
                nc.sync.dma_start(
                    out=v_sb[0:64, g0:g1],
                    in_=v_sb[64:128, 16 + g0:16 + g1],
                )
                nc.sync.dma_start(
                    out=v_sb[64:128, g0:g1],
                    in_=v_sb[0:64, 17 + g0:17 + g1],
                )

            def emit_scores_pair(c4, hp, pair):
                # scores + exp for both heads, one ii-pair (256 queries).
                # hh=0 lives in PE rows 0-63, hh=1 in rows 64-127: the
                # half-interleaved order lets consecutive matmuls execute
                # concurrently in disjoint row-groups. Both heads' exp
                # results land in one combined SBUF tile so the band mask
                # multiply is a single DVE op (mask broadcast over hh).
                if c4 == 0 and pair == 0:
                    mslice = masks_sb[:, :, 0:2]      # (first, interior)
                elif c4 == 3 and pair == 1:
                    mslice = masks_sb[:, :, 1:3]      # (interior, last)
                else:
                    mslice = masks_sb[:, :, 1:2]      # interior (iw-bcast)
                # one 2-bank tile for both heads: hh0 regions in bank A,
                # hh1 in bank B; a single exp ACTIVATE then covers the
                # whole unit (halves ACT per-op overhead)
                sct = pssc.tile([128, 2, 2, 2, 128], F32, tag="pssc",
                                name="sc")
                scr = sct[:].rearrange("p h a b c -> p (h a b) c")
                i0 = c4 * 4 + pair * 2
                if 1 <= i0 - 1 and i0 + 1 <= 14:
                    # interior pair: chunk i0 is shared by both query
                    # blocks (as cB of i0 and cA of i0+1) and its two
                    # score regions (iw0,half1)+(iw1,half0) are adjacent
                    # in the psum tile, so the pair needs only 3 matmuls
                    # per head: 128 + 256 + 128 columns
                    plan = [
                        (OFFS[i0 - 1], i0 * 128, 128, (0, 1)),
                        (OFFS[i0], i0 * 128, 256, (1, 3)),
                        (OFFS[i0 + 1], (i0 + 1) * 128, 128, (3, 4)),
                    ]
                else:
                    plan = []
                    for iw in range(2):
                        cA, cB = _chunk_pair(i0 + iw)
                        for half, cc in enumerate((cA, cB)):
                            plan.append((OFFS[cc], (i0 + iw) * 128, 128,
                                         (iw * 2 + half, iw * 2 + half + 1)))
                for mi, (koff, qoff, qn, (r0, r1)) in enumerate(plan):
                    for hh in range(2):
                        po = hh * 64
                        nc.tensor.matmul(
                            out=scr[:, 4 * hh + r0:4 * hh + r1, :],
                            lhsT=qk_sb[po:po + 64, 2 + hp,
                                       koff:koff + 128],
                            rhs=qk_sb[po:po + 64, hp, qoff:qoff + qn],
                            start=(mi == 0),
                            stop=(mi == len(plan) - 1),
                        )
                ex = epool.tile([128, 2, 2, 2, 128], F16, tag="exp",
                                name="ex")
                nc.scalar.activation(
                    out=ex[:],
                    in_=sct[:],
                    func=mybir.ActivationFunctionType.Exp,
                    bias=negc_sb[:],
                )
                nc.vector.tensor_mul(
                    out=ex[:],
                    in0=ex[:],
                    in1=mslice.broadcast_to([128, 2, 2, 2, 128]),
                )
                return ex

            def emit_av_norm_block(c4, hp, exs):
                # denominator-broadcast first: one ones-matmul per
                # (hh, pair, half) sums exp over keys straight from the ex
                # SBUF tile into bc rows 64*hh..64*hh+64 — the denominator
                # arrives already broadcast, so the reciprocal runs on DVE
                # concurrently with the AV matmuls.
                # each disjoint psum region's first write must carry
                # start=True: the has_written clear does not cover
                # partitions/regions the starting matmul doesn't write
                bc = ps512.tile([128, 512], F32, tag="ps512")
                for hh in range(2):
                    for pair in range(2):
                        for half in range(2):
                            nc.tensor.matmul(
                                out=bc[64 * hh:64 * hh + 64,
                                       256 * pair:256 * pair + 256],
                                lhsT=ones_sb[:],
                                rhs=exs[pair][:, hh, :, half, :],
                                start=(half == 0),
                                stop=(half == 1),
                            )
                bcs = wpool.tile([128, 512], F32, tag="bcs")
                nc.vector.reciprocal_approx_fast(out=bcs[:], in_=bc[:])

                # ii-outer so each 256-query half of the block finalizes
                # early; the per-pair aoT muls then run on DVE while the
                # second half's AV matmuls still stream on the PE
                avt = psav.tile([128, 4, 128], F32, tag="psav")
                for ii in range(4):
                    pair, iw = divmod(ii, 2)
                    cA, cB = _chunk_pair(c4 * 4 + ii)
                    for half, cc in enumerate((cA, cB)):
                        for hh in range(2):
                            nc.tensor.matmul(
                                out=avt[64 * hh:64 * hh + 64, ii, :],
                                lhsT=v_sb[:, VSLOT[cc], 2 * hp + hh, :],
                                rhs=exs[pair][:, hh, iw, half, :],
                                start=(half == 0),
                                stop=(half == 1),
                            )
                for pair in range(2):
                    sl = slice(c4 * 512 + 256 * pair,
                               c4 * 512 + 256 * pair + 256)
                    csl = slice(2 * pair, 2 * pair + 2)
                    nc.vector.tensor_mul(
                        out=aoT_sb[:, hp, sl],
                        in0=avt[:, csl, :].rearrange("p a b -> p (a b)"),
                        in1=bcs[:, 256 * pair:256 * pair + 256],
                    )

            yslot = {}

            def emit_outproj_st(st):
                # output staged in 2-st slots: one 512 KB DMA per slot with
                # 2 KB-contiguous DRAM rows (vs per-(st,nn) 128 KB DMAs
                # whose 1 KB rows ran the output path at ~100 GB/s and
                # backpressured ACT through the ysb pool)
                if st >= 14:
                    g, sub = st, 0  # final tiles ship solo to trim the tail
                else:
                    g, sub = divmod(st, 2)
                if sub == 0:
                    nst = 1 if st >= 14 else 2
                    yslot[g] = ypool.tile([128, nst, 1024], F16, tag="ysb",
                                          name="ysb")
                for nn in range(2):
                    # after the last scores unit the pssc banks are free:
                    # alternate the tail's outproj PSUM between pssc and
                    # ps512 so the final o-units stop stalling on the
                    # 2-deep ps512 ring
                    if st >= 12 and nn == 0:
                        ps = pssc.tile([128, 512], F32, tag="pssc",
                                       name="otail")
                    else:
                        ps = ps512.tile([128, 512], F32, tag="ps512")
                    for hp2 in range(2):
                        nc.tensor.matmul(
                            out=ps[:],
                            lhsT=aoT_sb[:, hp2, st * 128:(st + 1) * 128],
                            rhs=wout_sb[:, hp2, nn * 512:(nn + 1) * 512],
                            start=(hp2 == 0),
                            stop=(hp2 == 1),
                        )
                    # alternate the PSUM->SBUF copies over ACT/DVE so the
                    # two copies of one st run concurrently instead of
                    # serializing on ACT (which also owns the exps)
                    if nn == 0:
                        nc.scalar.copy(
                            out=yslot[g][:, sub, nn * 512:(nn + 1) * 512],
                            in_=ps[:],
                        )
                    else:
                        nc.vector.tensor_copy(
                            out=yslot[g][:, sub, nn * 512:(nn + 1) * 512],
                            in_=ps[:],
                        )
                if sub == 1 or st >= 14:
                    r0 = g * 256 if st < 14 else st * 128
                    r1 = r0 + (256 if st < 14 else 128)
                    nc.sync.dma_start(
                        out=out_d[r0:r1, :].rearrange(
                            "(s p) n -> p s n", p=128),
                        in_=yslot.pop(g)[:],
                    )

            # ---- emission schedule: per-pair scores -> filler -> AV so
            # the PE has independent work while ACT runs exp; projections
            # and outproj tiles are the fillers ----
            qk0_tiles = emit_qk_ns0_ktouter()

            # Schedule: tokens are ("s", c4, hp, pair) scores units,
            # ("a", c4, hp) per-block AV+norm, and fillers ("v", c) /
            # ("o", st) / ("qk", ns). Scores of pair-0 units for c4=0 only
            # need qk-ns0, so they run before QK1 (whose xT lands later).
            # va0 fills the PE while qk0's (m0, m2) bias ops run; the
            # (m1, m3) ops land after the first scores unit so it does not
            # falsely wait on them.
            SEQ = [
                ("va", 0),
                ("s", 0, 0, 0),
                ("qk0b",),
                ("qk", 1),
                ("s", 0, 1, 0), ("va", 1), ("va", 2), ("va", 3),
                ("s", 0, 0, 1), ("va", 4), ("sh", 0, 4), ("va", 5),
                ("a", 0, 0),
                ("s", 0, 1, 1), ("va", 6), ("a", 0, 1),
                ("qk", 2),
                ("s", 1, 0, 0), ("va", 7), ("va", 8), ("sh", 4, 8),
                ("s", 1, 0, 1), ("o", 0), ("a", 1, 0),
                ("s", 1, 1, 0), ("o", 1),
                ("s", 1, 1, 1), ("va", 9), ("a", 1, 1),
                ("qk", 3),
                ("s", 2, 0, 0), ("va", 10), ("va", 11),
                ("s", 2, 0, 1), ("va", 12), ("sh", 8, 12), ("o", 2),
                ("a", 2, 0),
                ("s", 2, 1, 0), ("o", 3), ("o", 4),
                ("s", 2, 1, 1), ("va", 13), ("o", 5), ("a", 2, 1),
                ("va", 14), ("va", 15), ("sh", 12, 15),
                ("s", 3, 0, 0), ("o", 6), ("o", 7),
                ("s", 3, 0, 1), ("o", 8), ("a", 3, 0),
                ("s", 3, 1, 0), ("o", 9), ("o", 10),
                ("s", 3, 1, 1), ("o", 11), ("a", 3, 1),
                ("o", 12), ("o", 13), ("o", 14), ("o", 15),
            ]

            exmap = {}
            for tok in SEQ:
                if tok[0] == "s":
                    _, c4, hp, pair = tok
                    exmap[(c4, hp, pair)] = emit_scores_pair(c4, hp, pair)
                elif tok[0] == "a":
                    _, c4, hp = tok
                    emit_av_norm_block(
                        c4, hp,
                        (exmap.pop((c4, hp, 0)), exmap.pop((c4, hp, 1))),
                    )
                elif tok[0] == "qk0b":
                    _qk0_ts(qk0_tiles, (1, 3))
                elif tok[0] == "va":
                    emit_v_aligned(tok[1])
                elif tok[0] == "sh":
                    emit_vshift(tok[1], tok[2])
                elif tok[0] == "o":
                    emit_outproj_st(tok[1])
                else:
                    emit_qk_ns(tok[1])

    nc.compile()
    return nc


_NC = None


def _get_program():
    global _NC
    if _NC is None:
        _NC = _build_program()
    return _NC


def _make_in_maps(x, w_qkv, b_qkv, w_out):
    masks = _build_masks()

    in_maps = []
    for c in range(8):
        b, hg = divmod(c, 4)
        cq = 256 * hg
        wqk = np.concatenate(
            [w_qkv[:, cq:cq + 256], w_qkv[:, 1024 + cq:1024 + cq + 256]],
            axis=1,
        ).astype(np.float16)
        # pre-rearrange to [kp, ko, n] so device DMA rows are contiguous
        wqk = np.ascontiguousarray(
            wqk.reshape(8, 128, 512).transpose(1, 0, 2))
        wv = np.ascontiguousarray(
            w_qkv[:, 2048 + cq:2048 + cq + 256].astype(np.float16)
            .reshape(8, 128, 256).transpose(1, 0, 2))
        bqk = np.empty((128, 4), np.float32)
        bqk[:, 0] = b_qkv[cq:cq + 128] * SCALE
        bqk[:, 1] = b_qkv[cq + 128:cq + 256] * SCALE
        bqk[:, 2] = b_qkv[1024 + cq:1024 + cq + 128]
        bqk[:, 3] = b_qkv[1024 + cq + 128:1024 + cq + 256]
        in_maps.append({
            "xT": np.ascontiguousarray(x[b].T).astype(np.float16),
            "wqk": wqk,
            "wv": wv,
            "wout": w_out[cq:cq + 256, :].astype(np.float16),
            "bqk": bqk,
            "masks": masks,
        })
    return in_maps


def kernel(x, w_qkv, b_qkv, w_out, b_out):
    x = np.asarray(x, np.float32)
    w_qkv = np.asarray(w_qkv, np.float32)
    b_qkv = np.asarray(b_qkv, np.float32)
    w_out = np.asarray(w_out, np.float32)
    b_out = np.asarray(b_out, np.float32)

    in_maps = _make_in_maps(x, w_qkv, b_qkv, w_out)
    nc = _get_program()
    res = run_bass_kernel_spmd(nc, in_maps, list(range(8)))

    b_v = b_qkv[2048:]
    bias_all = b_out + b_v @ w_out  # folds the (untracked) v-bias
    y = np.empty((B, S, D), np.float32)
    for b in range(B):
        acc = np.zeros((S, D), np.float32)
        for hg in range(4):
            acc += res.results[4 * b + hg]["out"].astype(np.float32)
        y[b] = acc + bias_all
    return y



# revision 59
# speedup vs baseline: 1.1811x; 1.1811x over previous
"""Trainium2 Bass kernel for LocalSparseAttention.

Problem (hardcoded): B=2, S=2048, D=1024, H=16, HD=64, WINDOW=128 (band
|i-j| <= 64), fp32 I/O.

Sharding: 8 cores = 2 batches x 4 head-groups (4 heads each). Each core:
  - qk projection into transposed layout [512, 2048] (head-pair packed)
  - v projection: 16 column-aligned seq chunks on the PE; the 15
    64-shifted chunks the banded attention needs are assembled by
    SBUF->SBUF partition-shift DMAs on the idle mid-kernel queue
  - banded attention, per-pair (256-query) scores with the two heads'
    K=64 matmuls interleaved (they co-execute in disjoint PE row-groups),
    exp on ACT into a combined per-pair tile, band mask as one DVE
    multiply (mask variants selected via broadcast APs from a compact
    [128,3,2,128] table), then per-block AV + normalization: a
    ones-matmul reads the exp tile directly and lands the softmax
    denominator PRE-BroADCAST in PSUM (denbc), so the reciprocal runs on
    DVE concurrently with the AV matmuls
  - output projection staged in 2-st slots -> one 512KB DMA per slot
Host: fp16 casts/transposes in, sum of 4 partials per batch + fused bias
(b_out + b_v @ w_out) out.

Schedule notes:
  - input DMAs carved (kt, ns)-wise, useful-first, first feed split
    across the Sync+ACT HWDGE queues; ~14 warmup matmuls on zeroed SBUF
    bridge the HAM clock-gate to the first feed's landing (the per-core
    DMA start jitters by ~1-2us; scoring is max-core)
  - qk0's (m1, m3) bias ops are deferred past the first scores unit:
    dependency tracking is tile-granular, so emitting them earlier would
    stall the unit on bias ops it does not read
  - every disjoint PSUM region's first matmul carries start=True: the
    hardware's has_written clear does NOT cover partitions the starting
    matmul doesn't write (stale bits => silent accumulation corruption)
  - emission order == per-engine execution order: each scores unit is
    followed by independent filler work (projection chunks / outproj
    tiles) sized to cover the exp+mask latency

All matmuls run in fp16 (1 cycle/row on PE warm at 2.4GHz) with fp32
PSUM accumulation; softmax exp input stays fp32. Interior score pairs
merge the shared key chunk into one 256-col matmul (3 matmuls/head
instead of 4). Outproj PSUM->SBUF copies alternate ACT/DVE to
unserialize the tail; the last 4 outproj units draw PSUM from the
then-idle scores pool. qk bias ops split DVE/ACT per chunk. Measured
96.5-97.8us max-core / ~95.8-96.4us mean over repeat runs (prev
session 103.2us; run-to-run max-core jitter ~1us), rel err 7.7e-4. fp8 (DoubleRow) for the
softmax-denominator matmul was tried and rejected: attention is peaked
enough that e4m3 quantization of exp does not average out
(rel err 1.8e-2, at the gate).
"""
import sys

if "/opt/trn_rl_repo" not in sys.path:
    sys.path.insert(0, "/opt/trn_rl_repo")

import numpy as np

import concourse.bass as bass
import concourse.mybir as mybir
import concourse.tile as tile
from concourse import bacc
from concourse.bass_utils import run_bass_kernel_spmd

B, S, D, H, HD = 2, 2048, 1024, 16, 64
SCALE = HD**-0.5
C_SUB = 4.0  # subtracted from all scores via the mask; cancels in softmax
MASK_NEG = -30000.0

F16 = mybir.dt.float16
F32 = mybir.dt.float32
F32R = mybir.dt.float32r
WARMUP_MM = 14

# 19 key/value chunk offsets: 15 shifted (128c+64) + aligned 0,128,1792,1920
OFFS = [128 * c + 64 for c in range(15)] + [0, 128, 1792, 1920]
# v_sb slot per chunk id: shifted c -> slot c; aligned 128a -> slot 16+a
VSLOT = {c: c for c in range(15)}
VSLOT.update({15: 16, 16: 17, 17: 30, 18: 31})


def _chunk_pair(i):
    if i == 0:
        return 15, 16
    if i == 15:
        return 17, 18
    return i - 1, i


def _build_pair_masks():
    # variant 0: (first, interior) — c4=0 pair 0
    # variant 1: (interior, interior)
    # variant 2: (interior, last)  — c4=3 pair 1
    m = _build_masks()  # [128, 3(first/int/last), 2(half), 128]
    mp = np.zeros((128, 3, 2, 2, 128), np.float16)
    mp[:, 0, 0] = m[:, 0]
    mp[:, 0, 1] = m[:, 1]
    mp[:, 1, 0] = m[:, 1]
    mp[:, 1, 1] = m[:, 1]
    mp[:, 2, 0] = m[:, 1]
    mp[:, 2, 1] = m[:, 2]
    return mp


def _build_masks():
    kp = np.arange(128)[:, None]
    p = np.arange(128)[None, :]
    masks = np.zeros((128, 3, 2, 128), np.float16)
    for v, shift in enumerate([0, 64, 128]):
        for half in (0, 1):
            w = 128 * half + kp
            valid = np.abs(p + shift - w) <= 64
            masks[:, v, half, :] = valid.astype(np.float16)
    return masks


def _build_program():
    nc = bacc.Bacc("TRN2", debug=False, num_devices=8)

    xT_d = nc.dram_tensor("xT", [D, S], F16, kind="ExternalInput").ap()
    # wqk/wv arrive host-rearranged to [kp, ko, n] so their DMA rows are
    # contiguous 2KB/4KB lines instead of the 1KB/512B strided rows a
    # device-side rearrange view would produce (input stream is the
    # binding constraint for the first ~25us)
    wqk_d = nc.dram_tensor("wqk", [128, 8, 512], F16,
                           kind="ExternalInput").ap()
    wv_d = nc.dram_tensor("wv", [128, 8, 256], F16,
                          kind="ExternalInput").ap()
    wout_d = nc.dram_tensor("wout", [256, D], F16, kind="ExternalInput").ap()
    bqk_d = nc.dram_tensor("bqk", [128, 4], F32, kind="ExternalInput").ap()
    masks_d = nc.dram_tensor("masks", [128, 3, 2, 128], F16,
                             kind="ExternalInput").ap()
    out_d = nc.dram_tensor("out", [S, D], F16, kind="ExternalOutput").ap()

    with tile.TileContext(nc) as tc:
        with (
            tc.tile_pool(name="const", bufs=1) as cpool,
            tc.tile_pool(name="work", bufs=3) as wpool,
            tc.tile_pool(name="expp", bufs=6) as epool,
            tc.tile_pool(name="ysb", bufs=5) as ypool,
            tc.tile_pool(name="ps512", bufs=2, space="PSUM") as ps512,
            tc.tile_pool(name="psv", bufs=1, space="PSUM") as psv,
            tc.tile_pool(name="pssc", bufs=2, space="PSUM") as pssc,
            tc.tile_pool(name="psav", bufs=1, space="PSUM") as psav,
        ):
            # ---- persistent SBUF tensors ----
            xT_sb = cpool.tile([128, 8, S], F16, tag="xT")
            wqk_sb = cpool.tile([128, 8, 512], F16, tag="wqk")
            wv_sb = cpool.tile([128, 8, 256], F16, tag="wv")
            wout_sb = cpool.tile([128, 2, D], F16, tag="wout")
            bqk_sb = cpool.tile([128, 4], F32, tag="bqk")
            masks_sb = cpool.tile([128, 1, 3, 2, 128], F16, tag="masks")
            qk_sb = cpool.tile([128, 4, S], F16, tag="qk")
            v_sb = cpool.tile([128, 32, 4, 64], F16, tag="v")
            aoT_sb = cpool.tile([128, 2, S], F16, tag="aoT")
            ones_sb = cpool.tile([128, 64], F16, tag="ones")
            negc_sb = cpool.tile([128, 1], F32, tag="negc")

            xT_r = xT_d.rearrange("(ko kp) s -> kp ko s", kp=128)
            wqk_r = wqk_d
            wv_r = wv_d

            # ---- input DMAs ----
            # First feed split across the two HWDGE queues so the two
            # halves stream concurrently: wqk[0:2] on Sync, its xT mate on
            # the ACT queue (idle early). bqk follows on ACT (needed only
            # at the first qk TS ~19us).
            nc.scalar.dma_start(out=xT_sb[:, 0:2, 0:512],
                                in_=xT_r[:, 0:2, 0:512])
            nc.scalar.dma_start(out=bqk_sb[:], in_=bqk_d)
            # Sync queue, priority order, kt-pairwise so qk0's kt-outer
            # loop can start each kt slice as soon as its pair lands.
            nc.sync.dma_start(out=wqk_sb[:, 0:2], in_=wqk_r[:, 0:2])
            for k0 in (2, 4, 6):
                nc.sync.dma_start(out=wqk_sb[:, k0:k0 + 2],
                                  in_=wqk_r[:, k0:k0 + 2])
                nc.sync.dma_start(out=xT_sb[:, k0:k0 + 2, 0:512],
                                  in_=xT_r[:, k0:k0 + 2, 0:512])
            nc.sync.dma_start(out=wv_sb[:], in_=wv_r[:])
            nc.sync.dma_start(out=xT_sb[:, :, 512:1024],
                              in_=xT_r[:, :, 512:1024])
            nc.sync.dma_start(out=masks_sb[:, 0], in_=masks_d)
            nc.sync.dma_start(out=xT_sb[:, :, 1024:1536],
                              in_=xT_r[:, :, 1024:1536])
            nc.sync.dma_start(out=xT_sb[:, :, 1536:2048],
                              in_=xT_r[:, :, 1536:2048])
            nc.sync.dma_start(
                out=wout_sb[:],
                in_=wout_d.rearrange("(t p) n -> p t n", p=128),
            )

            wsrc = cpool.tile([128, 256], F16, tag="wsrc")
            nc.vector.memset(wsrc[:], 0.0)
            nc.vector.memset(ones_sb[:], 1.0)
            nc.vector.memset(negc_sb[:], -C_SUB)

            # ---- PE warmup: dummy matmuls on zeroed SBUF bridge the HAM
            # clock-gate ramp until the first feed's landing (~8.3us with
            # the split-queue first feed) ----
            wps = psv.tile([128, 256], F32, tag="psv")
            for w in range(WARMUP_MM):
                nc.tensor.matmul(
                    out=wps[:],
                    lhsT=wsrc[:, 0:128],
                    rhs=wsrc[:],
                    start=(w == 0),
                    stop=(w == WARMUP_MM - 1),
                )
            wdst = wpool.tile([128, 256], F16, tag="wdst")
            nc.scalar.copy(out=wdst[:], in_=wps[:])

            # ---- emission helpers ----
            def _qk0_ts(tiles, ms):
                # bias+scale split across DVE (m=0,1) and ACT (m=2,3) so
                # the pair a scores unit needs runs on two engines at once
                for m in ms:
                    scale = SCALE if m < 2 else 1.0
                    if m < 2:
                        nc.vector.tensor_scalar(
                            out=qk_sb[:, m, 0:512],
                            in0=tiles[m][:],
                            scalar1=scale,
                            scalar2=bqk_sb[:, m:m + 1],
                            op0=mybir.AluOpType.mult,
                            op1=mybir.AluOpType.add,
                        )
                    else:
                        nc.scalar.activation(
                            out=qk_sb[:, m, 0:512],
                            in_=tiles[m][:],
                            func=mybir.ActivationFunctionType.Identity,
                            bias=bqk_sb[:, m:m + 1],
                            scale=scale,
                        )

            def emit_qk_ns0_ktouter():
                # first qk chunk, kt-outer so matmuls start as soon as the
                # first (wqk, xT) kt-slices land; 4 psum banks open at
                # once. m-order (0,2,1,3) so the hp=0 pair's PSUM closes
                # first. Only the (m0, m2) bias ops are emitted here — the
                # (m1, m3) ops are deferred past the first scores unit
                # (tile-granular dependency tracking would otherwise make
                # it wait on all four).
                rrp = [(ps512, "ps512"), (pssc, "pssc"), (psav, "psav"),
                       (ps512, "ps512")]
                tiles = [pool.tile([128, 512], F32, tag=tg, name=f"qk0m{m}")
                         for m, (pool, tg) in enumerate(rrp)]
                for kt in range(8):
                    for m in (0, 2, 1, 3):
                        nc.tensor.matmul(
                            out=tiles[m][:],
                            lhsT=wqk_sb[:, kt, m * 128:(m + 1) * 128],
                            rhs=xT_sb[:, kt, 0:512],
                            start=(kt == 0),
                            stop=(kt == 7),
                        )
                _qk0_ts(tiles, (0, 2))
                return tiles

            def emit_qk_ns(ns):
                for m in (0, 2, 1, 3):
                    scale = SCALE if m < 2 else 1.0
                    ps = ps512.tile([128, 512], F32, tag="ps512")
                    for kt in range(8):
                        nc.tensor.matmul(
                            out=ps[:],
                            lhsT=wqk_sb[:, kt, m * 128:(m + 1) * 128],
                            rhs=xT_sb[:, kt, ns * 512:(ns + 1) * 512],
                            start=(kt == 0),
                            stop=(kt == 7),
                        )
                    # bias+scale on DVE for m=0,1 and ACT for m=2,3 so the
                    # (q, k) pair a scores unit reads finishes ~2x sooner
                    if m < 2:
                        nc.vector.tensor_scalar(
                            out=qk_sb[:, m, ns * 512:(ns + 1) * 512],
                            in0=ps[:],
                            scalar1=scale,
                            scalar2=bqk_sb[:, m:m + 1],
                            op0=mybir.AluOpType.mult,
                            op1=mybir.AluOpType.add,
                        )
                    else:
                        nc.scalar.activation(
                            out=qk_sb[:, m, ns * 512:(ns + 1) * 512],
                            in_=ps[:],
                            func=mybir.ActivationFunctionType.Identity,
                            bias=bqk_sb[:, m:m + 1],
                            scale=scale,
                        )

            def emit_v_aligned(a):
                # aligned v chunk a covers seq [128a, 128a+128); the
                # 64-shifted chunks are assembled later by partition-shift
                # DMAs instead of redundant matmuls
                off = 128 * a
                ps = psv.tile([128, 256], F32, tag="psv")
                for kt in range(8):
                    nc.tensor.matmul(
                        out=ps[:],
                        lhsT=xT_sb[:, kt, off:off + 128],
                        rhs=wv_sb[:, kt, :],
                        start=(kt == 0),
                        stop=(kt == 7),
                    )
                nc.scalar.copy(
                    out=v_sb[:, 16 + a],
                    in_=ps[:].rearrange("p (h d) -> p h d", h=4),
                )

            def emit_vshift(g0, g1):
                # shifted chunk c = aligned(c) rows 64:128 ++ aligned(c+1)
                # rows 0:64; two strided SBUF->SBUF DMAs build chunks
                # [g0, g1) from aligned slots on the idle mid-kernel queue
                nc.sync.dma_start(
                    out=v_sb[0:64, g0:g1],
                    in_=v_sb[64:128, 16 + g0:16 + g1],
                )
                nc.sync.dma_start(
                    out=v_sb[64:128, g0:g1],
                    in_=v_sb[0:64, 17 + g0:17 + g1],
                )

            def emit_scores_pair(c4, hp, pair):
                # scores + exp for both heads, one ii-pair (256 queries).
                # hh=0 lives in PE rows 0-63, hh=1 in rows 64-127: the
                # half-interleaved order lets consecutive matmuls execute
                # concurrently in disjoint row-groups. Both heads' exp
                # results land in one combined SBUF tile so the band mask
                # multiply is a single DVE op (mask broadcast over hh).
                if c4 == 0 and pair == 0:
                    mslice = masks_sb[:, :, 0:2]      # (first, interior)
                elif c4 == 3 and pair == 1:
                    mslice = masks_sb[:, :, 1:3]      # (interior, last)
                else:
                    mslice = masks_sb[:, :, 1:2]      # interior (iw-bcast)
                # one 2-bank tile for both heads: hh0 regions in bank A,
                # hh1 in bank B; a single exp ACTIVATE then covers the
                # whole unit (halves ACT per-op overhead)
                sct = pssc.tile([128, 2, 2, 2, 128], F32, tag="pssc",
                                name="sc")
                scr = sct[:].rearrange("p h a b c -> p (h a b) c")
                i0 = c4 * 4 + pair * 2
                if 1 <= i0 - 1 and i0 + 1 <= 14:
                    # interior pair: chunk i0 is shared by both query
                    # blocks (as cB of i0 and cA of i0+1) and its two
                    # score regions (iw0,half1)+(iw1,half0) are adjacent
                    # in the psum tile, so the pair needs only 3 matmuls
                    # per head: 128 + 256 + 128 columns
                    plan = [
                        (OFFS[i0 - 1], i0 * 128, 128, (0, 1)),
                        (OFFS[i0], i0 * 128, 256, (1, 3)),
                        (OFFS[i0 + 1], (i0 + 1) * 128, 128, (3, 4)),
                    ]
                else:
                    plan = []
                    for iw in range(2):
                        cA, cB = _chunk_pair(i0 + iw)
                        for half, cc in enumerate((cA, cB)):
                            plan.append((OFFS[cc], (i0 + iw) * 128, 128,
                                         (iw * 2 + half, iw * 2 + half + 1)))
                for mi, (koff, qoff, qn, (r0, r1)) in enumerate(plan):
                    for hh in range(2):
                        po = hh * 64
                        nc.tensor.matmul(
                            out=scr[:, 4 * hh + r0:4 * hh + r1, :],
                            lhsT=qk_sb[po:po + 64, 2 + hp,
                                       koff:koff + 128],
                            rhs=qk_sb[po:po + 64, hp, qoff:qoff + qn],
                            start=(mi == 0),
                            stop=(mi == len(plan) - 1),
                        )
                ex = epool.tile([128, 2, 2, 2, 128], F16, tag="exp",
                                name="ex")
                nc.scalar.activation(
                    out=ex[:],
                    in_=sct[:],
                    func=mybir.ActivationFunctionType.Exp,
                    bias=negc_sb[:],
                )
                nc.vector.tensor_mul(
                    out=ex[:],
                    in0=ex[:],
                    in1=mslice.broadcast_to([128, 2, 2, 2, 128]),
                )
                return ex

            def emit_av_norm_block(c4, hp, exs):
                # denominator-broadcast first: one ones-matmul per
                # (hh, pair, half) sums exp over keys straight from the ex
                # SBUF tile into bc rows 64*hh..64*hh+64 — the denominator
                # arrives already broadcast, so the reciprocal runs on DVE
                # concurrently with the AV matmuls.
                # each disjoint psum region's first write must carry
                # start=True: the has_written clear does not cover
                # partitions/regions the starting matmul doesn't write
                bc = ps512.tile([128, 512], F32, tag="ps512")
                for hh in range(2):
                    for pair in range(2):
                        for half in range(2):
                            nc.tensor.matmul(
                                out=bc[64 * hh:64 * hh + 64,
                                       256 * pair:256 * pair + 256],
                                lhsT=ones_sb[:],
                                rhs=exs[pair][:, hh, :, half, :],
                                start=(half == 0),
                                stop=(half == 1),
                            )
                bcs = wpool.tile([128, 512], F32, tag="bcs")
                nc.vector.reciprocal_approx_fast(out=bcs[:], in_=bc[:])

                # ii-outer so each 256-query half of the block finalizes
                # early; the per-pair aoT muls then run on DVE while the
                # second half's AV matmuls still stream on the PE
                avt = psav.tile([128, 4, 128], F32, tag="psav")
                for ii in range(4):
                    pair, iw = divmod(ii, 2)
                    cA, cB = _chunk_pair(c4 * 4 + ii)
                    for half, cc in enumerate((cA, cB)):
                        for hh in range(2):
                            nc.tensor.matmul(
                                out=avt[64 * hh:64 * hh + 64, ii, :],
                                lhsT=v_sb[:, VSLOT[cc], 2 * hp + hh, :],
                                rhs=exs[pair][:, hh, iw, half, :],
                                start=(half == 0),
                                stop=(half == 1),
                            )
                for pair in range(2):
                    sl = slice(c4 * 512 + 256 * pair,
                               c4 * 512 + 256 * pair + 256)
                    csl = slice(2 * pair, 2 * pair + 2)
                    nc.vector.tensor_mul(
                        out=aoT_sb[:, hp, sl],
                        in0=avt[:, csl, :].rearrange("p a b -> p (a b)"),
                        in1=bcs[:, 256 * pair:256 * pair + 256],
                    )

            yslot = {}

            def emit_outproj_st(st):
                # output staged in 2-st slots: one 512 KB DMA per slot with
                # 2 KB-contiguous DRAM rows (vs per-(st,nn) 128 KB DMAs
                # whose 1 KB rows ran the output path at ~100 GB/s and
                # backpressured ACT through the ysb pool)
                if st >= 14:
                    g, sub = st, 0  # final tiles ship solo to trim the tail
                else:
                    g, sub = divmod(st, 2)
                if sub == 0:
                    nst = 1 if st >= 14 else 2
                    yslot[g] = ypool.tile([128, nst, 1024], F16, tag="ysb",
                                          name="ysb")
                for nn in range(2):
                    # after the last scores unit the pssc banks are free:
                    # alternate the tail's outproj PSUM between pssc and
                    # ps512 so the final o-units stop stalling on the
                    # 2-deep ps512 ring
                    if st >= 12 and nn == 0:
                        ps = pssc.tile([128, 512], F32, tag="pssc",
                                       name="otail")
                    else:
                        ps = ps512.tile([128, 512], F32, tag="ps512")
                    for hp2 in range(2):
                        nc.tensor.matmul(
                            out=ps[:],
                            lhsT=aoT_sb[:, hp2, st * 128:(st + 1) * 128],
                            rhs=wout_sb[:, hp2, nn * 512:(nn + 1) * 512],
                            start=(hp2 == 0),
                            stop=(hp2 == 1),
                        )
                    # alternate the PSUM->SBUF copies over ACT/DVE so the
                    # two copies of one st run concurrently instead of
                    # serializing on ACT (which also owns the exps)
                    if nn == 0:
                        nc.scalar.copy(
                            out=yslot[g][:, sub, nn * 512:(nn + 1) * 512],
                            in_=ps[:],
                        )
                    else:
                        nc.vector.tensor_copy(
                            out=yslot[g][:, sub, nn * 512:(nn + 1) * 512],
                            in_=ps[:],
                        )
                if sub == 1 or st >= 14:
                    r0 = g * 256 if st < 14 else st * 128
                    r1 = r0 + (256 if st < 14 else 128)
                    nc.sync.dma_start(
                        out=out_d[r0:r1, :].rearrange(
                            "(s p) n -> p s n", p=128),
                        in_=yslot.pop(g)[:],
                    )

            # ---- emission schedule: per-pair scores -> filler -> AV so
            # the PE has independent work while ACT runs exp; projections
            # and outproj tiles are the fillers ----
            qk0_tiles = emit_qk_ns0_ktouter()

            # Schedule: tokens are ("s", c4, hp, pair) scores units,
            # ("a", c4, hp) per-block AV+norm, and fillers ("v", c) /
            # ("o", st) / ("qk", ns). Scores of pair-0 units for c4=0 only
            # need qk-ns0, so they run before QK1 (whose xT lands later).
            # va0 fills the PE while qk0's (m0, m2) bias ops run; the
            # (m1, m3) ops land after the first scores unit so it does not
            # falsely wait on them.
            SEQ = [
                ("va", 0),
                ("s", 0, 0, 0),
                ("qk0b",),
                ("qk", 1),
                ("s", 0, 1, 0), ("va", 1), ("va", 2), ("va", 3),
                ("s", 0, 0, 1), ("va", 4), ("sh", 0, 4), ("va", 5),
                ("a", 0, 0),
                ("s", 0, 1, 1), ("va", 6), ("a", 0, 1),
                ("qk", 2),
                ("s", 1, 0, 0), ("va", 7), ("va", 8), ("sh", 4, 8),
                ("s", 1, 0, 1), ("o", 0), ("a", 1, 0),
                ("s", 1, 1, 0), ("o", 1),
                ("s", 1, 1, 1), ("va", 9), ("a", 1, 1),
                ("qk", 3),
                ("s", 2, 0, 0), ("va", 10), ("va", 11),
                ("s", 2, 0, 1), ("va", 12), ("sh", 8, 12), ("o", 2),
                ("a", 2, 0),
                ("s", 2, 1, 0), ("o", 3), ("o", 4),
                ("s", 2, 1, 1), ("va", 13), ("o", 5), ("a", 2, 1),
                ("va", 14), ("va", 15), ("sh", 12, 15),
                ("s", 3, 0, 0), ("o", 6), ("o", 7),
                ("s", 3, 0, 1), ("o", 8), ("a", 3, 0),
                ("s", 3, 1, 0), ("o", 9), ("o", 10),
                ("s", 3, 1, 1), ("o", 11), ("a", 3, 1),
                ("o", 12), ("o", 13), ("o", 14), ("o", 15),
            ]

            exmap = {}
            for tok in SEQ:
                if tok[0] == "s":
                    _, c4, hp, pair = tok
                    exmap[(c4, hp, pair)] = emit_scores_pair(c4, hp, pair)
                elif tok[0] == "a":
                    _, c4, hp = tok
                    emit_av_norm_block(
                        c4, hp,
                        (exmap.pop((c4, hp, 0)), exmap.pop((c4, hp, 1))),
                    )
                elif tok[0] == "qk0b":
                    _qk0_ts(qk0_tiles, (1, 3))
                elif tok[0] == "va":
                    emit_v_aligned(tok[1])
                elif tok[0] == "sh":
                    emit_vshift(tok[1], tok[2])
                elif tok[0] == "o":
                    emit_outproj_st(tok[1])
                else:
                    emit_qk_ns(tok[1])

    nc.compile()
    return nc


_NC = None


def _get_program():
    global _NC
    if _NC is None:
        _NC = _build_program()
    return _NC


def _make_in_maps(x, w_qkv, b_qkv, w_out):
    masks = _build_masks()

    in_maps = []
    for c in range(8):
        b, hg = divmod(c, 4)
        cq = 256 * hg
        wqk = np.concatenate(
            [w_qkv[:, cq:cq + 256], w_qkv[:, 1024 + cq:1024 + cq + 256]],
            axis=1,
        ).astype(np.float16)
        # pre-rearrange to [kp, ko, n] so device DMA rows are contiguous
        wqk = np.ascontiguousarray(
            wqk.reshape(8, 128, 512).transpose(1, 0, 2))
        wv = np.ascontiguousarray(
            w_qkv[:, 2048 + cq:2048 + cq + 256].astype(np.float16)
            .reshape(8, 128, 256).transpose(1, 0, 2))
        bqk = np.empty((128, 4), np.float32)
        bqk[:, 0] = b_qkv[cq:cq + 128] * SCALE
        bqk[:, 1] = b_qkv[cq + 128:cq + 256] * SCALE
        bqk[:, 2] = b_qkv[1024 + cq:1024 + cq + 128]
        bqk[:, 3] = b_qkv[1024 + cq + 128:1024 + cq + 256]
        in_maps.append({
            "xT": np.ascontiguousarray(x[b].T).astype(np.float16),
            "wqk": wqk,
            "wv": wv,
            "wout": w_out[cq:cq + 256, :].astype(np.float16),
            "bqk": bqk,
            "masks": masks,
        })
    return in_maps


def kernel(x, w_qkv, b_qkv, w_out, b_out):
    x = np.asarray(x, np.float32)
    w_qkv = np.asarray(w_qkv, np.float32)
    b_qkv = np.asarray(b_qkv, np.float32)
    w_out = np.asarray(w_out, np.float32)
    b_out = np.asarray(b_out, np.float32)

    in_maps = _make_in_maps(x, w_qkv, b_qkv, w_out)
    nc = _get_program()
    res = run_bass_kernel_spmd(nc, in_maps, list(range(8)))

    b_v = b_qkv[2048:]
    bias_all = b_out + b_v @ w_out  # folds the (untracked) v-bias
    y = np.empty((B, S, D), np.float32)
    for b in range(B):
        acc = np.zeros((S, D), np.float32)
        for hg in range(4):
            acc += res.results[4 * b + hg]["out"].astype(np.float32)
        y[b] = acc + bias_all
    return y



# revision 60
# speedup vs baseline: 1.2144x; 1.0282x over previous
"""Trainium2 Bass kernel for LocalSparseAttention.

Problem (hardcoded): B=2, S=2048, D=1024, H=16, HD=64, WINDOW=128 (band
|i-j| <= 64), fp32 I/O.

Sharding: 8 cores = 2 batches x 4 head-groups (4 heads each). Each core:
  - qk projection into transposed layout [512, 2048] (head-pair packed)
  - v projection: 16 column-aligned seq chunks on the PE; the 15
    64-shifted chunks the banded attention needs are assembled by
    SBUF->SBUF partition-shift DMAs on the idle mid-kernel queue
  - banded attention, per-pair (256-query) scores with the two heads'
    K=64 matmuls interleaved (they co-execute in disjoint PE row-groups),
    exp on ACT into a combined per-pair tile, band mask as one DVE
    multiply (mask variants selected via broadcast APs from a compact
    [128,3,2,128] table), then per-block AV + normalization: a
    ones-matmul reads the exp tile directly and lands the softmax
    denominator PRE-BroADCAST in PSUM (denbc), so the reciprocal runs on
    DVE concurrently with the AV matmuls
  - output projection staged in 2-st slots -> one 512KB DMA per slot
Host: fp16 casts/transposes in, sum of 4 partials per batch + fused bias
(b_out + b_v @ w_out) out.

Schedule notes:
  - input DMAs carved (kt, ns)-wise, useful-first, first feed split
    across the Sync+ACT HWDGE queues; ~14 warmup matmuls on zeroed SBUF
    bridge the HAM clock-gate to the first feed's landing (the per-core
    DMA start jitters by ~1-2us; scoring is max-core)
  - qk0's (m1, m3) bias ops are deferred past the first scores unit:
    dependency tracking is tile-granular, so emitting them earlier would
    stall the unit on bias ops it does not read
  - every disjoint PSUM region's first matmul carries start=True: the
    hardware's has_written clear does NOT cover partitions the starting
    matmul doesn't write (stale bits => silent accumulation corruption)
  - emission order == per-engine execution order: each scores unit is
    followed by independent filler work (projection chunks / outproj
    tiles) sized to cover the exp+mask latency

All matmuls run in fp16 (1 cycle/row on PE warm at 2.4GHz) with fp32
PSUM accumulation; softmax exp input stays fp32. Interior score pairs
merge the shared key chunk into one 256-col matmul (3 matmuls/head
instead of 4). Outproj PSUM->SBUF copies alternate ACT/DVE to
unserialize the tail; the last 4 outproj units draw PSUM from the
then-idle scores pool. Measured ~97.8us max-core / ~96.8us mean
(prev session 103.2us), rel err 7.7e-4. fp8 (DoubleRow) for the
softmax-denominator matmul was tried and rejected: attention is peaked
enough that e4m3 quantization of exp does not average out
(rel err 1.8e-2, at the gate).
"""
import sys

if "/opt/trn_rl_repo" not in sys.path:
    sys.path.insert(0, "/opt/trn_rl_repo")

import numpy as np

import concourse.bass as bass
import concourse.mybir as mybir
import concourse.tile as tile
from concourse import bacc
from concourse.bass_utils import run_bass_kernel_spmd

B, S, D, H, HD = 2, 2048, 1024, 16, 64
SCALE = HD**-0.5
C_SUB = 4.0  # subtracted from all scores via the mask; cancels in softmax
MASK_NEG = -30000.0

F16 = mybir.dt.float16
F32 = mybir.dt.float32
F32R = mybir.dt.float32r
WARMUP_MM = 14

# 19 key/value chunk offsets: 15 shifted (128c+64) + aligned 0,128,1792,1920
OFFS = [128 * c + 64 for c in range(15)] + [0, 128, 1792, 1920]
# v_sb slot per chunk id: shifted c -> slot c; aligned 128a -> slot 16+a
VSLOT = {c: c for c in range(15)}
VSLOT.update({15: 16, 16: 17, 17: 30, 18: 31})


def _chunk_pair(i):
    if i == 0:
        return 15, 16
    if i == 15:
        return 17, 18
    return i - 1, i


def _build_pair_masks():
    # variant 0: (first, interior) — c4=0 pair 0
    # variant 1: (interior, interior)
    # variant 2: (interior, last)  — c4=3 pair 1
    m = _build_masks()  # [128, 3(first/int/last), 2(half), 128]
    mp = np.zeros((128, 3, 2, 2, 128), np.float16)
    mp[:, 0, 0] = m[:, 0]
    mp[:, 0, 1] = m[:, 1]
    mp[:, 1, 0] = m[:, 1]
    mp[:, 1, 1] = m[:, 1]
    mp[:, 2, 0] = m[:, 1]
    mp[:, 2, 1] = m[:, 2]
    return mp


def _build_masks():
    kp = np.arange(128)[:, None]
    p = np.arange(128)[None, :]
    masks = np.zeros((128, 3, 2, 128), np.float16)
    for v, shift in enumerate([0, 64, 128]):
        for half in (0, 1):
            w = 128 * half + kp
            valid = np.abs(p + shift - w) <= 64
            masks[:, v, half, :] = valid.astype(np.float16)
    return masks


def _build_program():
    nc = bacc.Bacc("TRN2", debug=False, num_devices=8)

    xT_d = nc.dram_tensor("xT", [D, S], F16, kind="ExternalInput").ap()
    wqk_d = nc.dram_tensor("wqk", [D, 512], F16, kind="ExternalInput").ap()
    wv_d = nc.dram_tensor("wv", [D, 256], F16, kind="ExternalInput").ap()
    wout_d = nc.dram_tensor("wout", [256, D], F16, kind="ExternalInput").ap()
    bqk_d = nc.dram_tensor("bqk", [128, 4], F32, kind="ExternalInput").ap()
    masks_d = nc.dram_tensor("masks", [128, 3, 2, 128], F16,
                             kind="ExternalInput").ap()
    out_d = nc.dram_tensor("out", [S, D], F16, kind="ExternalOutput").ap()

    with tile.TileContext(nc) as tc:
        with (
            tc.tile_pool(name="const", bufs=1) as cpool,
            tc.tile_pool(name="work", bufs=3) as wpool,
            tc.tile_pool(name="expp", bufs=6) as epool,
            tc.tile_pool(name="ysb", bufs=5) as ypool,
            tc.tile_pool(name="ps512", bufs=2, space="PSUM") as ps512,
            tc.tile_pool(name="psv", bufs=1, space="PSUM") as psv,
            tc.tile_pool(name="pssc", bufs=2, space="PSUM") as pssc,
            tc.tile_pool(name="psav", bufs=1, space="PSUM") as psav,
        ):
            # ---- persistent SBUF tensors ----
            xT_sb = cpool.tile([128, 8, S], F16, tag="xT")
            wqk_sb = cpool.tile([128, 8, 512], F16, tag="wqk")
            wv_sb = cpool.tile([128, 8, 256], F16, tag="wv")
            wout_sb = cpool.tile([128, 2, D], F16, tag="wout")
            bqk_sb = cpool.tile([128, 4], F32, tag="bqk")
            masks_sb = cpool.tile([128, 1, 3, 2, 128], F16, tag="masks")
            qk_sb = cpool.tile([128, 4, S], F16, tag="qk")
            v_sb = cpool.tile([128, 32, 4, 64], F16, tag="v")
            aoT_sb = cpool.tile([128, 2, S], F16, tag="aoT")
            ones_sb = cpool.tile([128, 64], F16, tag="ones")
            negc_sb = cpool.tile([128, 1], F32, tag="negc")

            xT_r = xT_d.rearrange("(ko kp) s -> kp ko s", kp=128)
            wqk_r = wqk_d.rearrange("(ko kp) n -> kp ko n", kp=128)
            wv_r = wv_d.rearrange("(ko kp) n -> kp ko n", kp=128)

            # ---- input DMAs ----
            # First feed split across the two HWDGE queues so the two
            # halves stream concurrently: wqk[0:2] on Sync, its xT mate on
            # the ACT queue (idle early). bqk follows on ACT (needed only
            # at the first qk TS ~19us).
            nc.scalar.dma_start(out=xT_sb[:, 0:2, 0:512],
                                in_=xT_r[:, 0:2, 0:512])
            nc.scalar.dma_start(out=bqk_sb[:], in_=bqk_d)
            # Sync queue, priority order, kt-pairwise so qk0's kt-outer
            # loop can start each kt slice as soon as its pair lands.
            nc.sync.dma_start(out=wqk_sb[:, 0:2], in_=wqk_r[:, 0:2])
            for k0 in (2, 4, 6):
                nc.sync.dma_start(out=wqk_sb[:, k0:k0 + 2],
                                  in_=wqk_r[:, k0:k0 + 2])
                nc.sync.dma_start(out=xT_sb[:, k0:k0 + 2, 0:512],
                                  in_=xT_r[:, k0:k0 + 2, 0:512])
            nc.sync.dma_start(out=wv_sb[:], in_=wv_r[:])
            nc.sync.dma_start(out=xT_sb[:, :, 512:1024],
                              in_=xT_r[:, :, 512:1024])
            nc.sync.dma_start(out=masks_sb[:, 0], in_=masks_d)
            nc.sync.dma_start(out=xT_sb[:, :, 1024:1536],
                              in_=xT_r[:, :, 1024:1536])
            nc.sync.dma_start(out=xT_sb[:, :, 1536:2048],
                              in_=xT_r[:, :, 1536:2048])
            nc.sync.dma_start(
                out=wout_sb[:],
                in_=wout_d.rearrange("(t p) n -> p t n", p=128),
            )

            wsrc = cpool.tile([128, 256], F16, tag="wsrc")
            nc.vector.memset(wsrc[:], 0.0)
            nc.vector.memset(ones_sb[:], 1.0)
            nc.vector.memset(negc_sb[:], -C_SUB)

            # ---- PE warmup: dummy matmuls on zeroed SBUF bridge the HAM
            # clock-gate ramp until the first feed's landing (~8.3us with
            # the split-queue first feed) ----
            wps = psv.tile([128, 256], F32, tag="psv")
            for w in range(WARMUP_MM):
                nc.tensor.matmul(
                    out=wps[:],
                    lhsT=wsrc[:, 0:128],
                    rhs=wsrc[:],
                    start=(w == 0),
                    stop=(w == WARMUP_MM - 1),
                )
            wdst = wpool.tile([128, 256], F16, tag="wdst")
            nc.scalar.copy(out=wdst[:], in_=wps[:])

            # ---- emission helpers ----
            def _qk0_ts(tiles, ms):
                # bias+scale split across DVE (m=0,1) and ACT (m=2,3) so
                # the pair a scores unit needs runs on two engines at once
                for m in ms:
                    scale = SCALE if m < 2 else 1.0
                    if m < 2:
                        nc.vector.tensor_scalar(
                            out=qk_sb[:, m, 0:512],
                            in0=tiles[m][:],
                            scalar1=scale,
                            scalar2=bqk_sb[:, m:m + 1],
                            op0=mybir.AluOpType.mult,
                            op1=mybir.AluOpType.add,
                        )
                    else:
                        nc.scalar.activation(
                            out=qk_sb[:, m, 0:512],
                            in_=tiles[m][:],
                            func=mybir.ActivationFunctionType.Identity,
                            bias=bqk_sb[:, m:m + 1],
                            scale=scale,
                        )

            def emit_qk_ns0_ktouter():
                # first qk chunk, kt-outer so matmuls start as soon as the
                # first (wqk, xT) kt-slices land; 4 psum banks open at
                # once. m-order (0,2,1,3) so the hp=0 pair's PSUM closes
                # first. Only the (m0, m2) bias ops are emitted here — the
                # (m1, m3) ops are deferred past the first scores unit
                # (tile-granular dependency tracking would otherwise make
                # it wait on all four).
                rrp = [(ps512, "ps512"), (pssc, "pssc"), (psav, "psav"),
                       (ps512, "ps512")]
                tiles = [pool.tile([128, 512], F32, tag=tg, name=f"qk0m{m}")
                         for m, (pool, tg) in enumerate(rrp)]
                for kt in range(8):
                    for m in (0, 2, 1, 3):
                        nc.tensor.matmul(
                            out=tiles[m][:],
                            lhsT=wqk_sb[:, kt, m * 128:(m + 1) * 128],
                            rhs=xT_sb[:, kt, 0:512],
                            start=(kt == 0),
                            stop=(kt == 7),
                        )
                _qk0_ts(tiles, (0, 2))
                return tiles

            def emit_qk_ns(ns):
                for m in (0, 2, 1, 3):
                    scale = SCALE if m < 2 else 1.0
                    ps = ps512.tile([128, 512], F32, tag="ps512")
                    for kt in range(8):
                        nc.tensor.matmul(
                            out=ps[:],
                            lhsT=wqk_sb[:, kt, m * 128:(m + 1) * 128],
                            rhs=xT_sb[:, kt, ns * 512:(ns + 1) * 512],
                            start=(kt == 0),
                            stop=(kt == 7),
                        )
                    # bias+scale on DVE for m=0,1 and ACT for m=2,3 so the
                    # (q, k) pair a scores unit reads finishes ~2x sooner
                    if m < 2:
                        nc.vector.tensor_scalar(
                            out=qk_sb[:, m, ns * 512:(ns + 1) * 512],
                            in0=ps[:],
                            scalar1=scale,
                            scalar2=bqk_sb[:, m:m + 1],
                            op0=mybir.AluOpType.mult,
                            op1=mybir.AluOpType.add,
                        )
                    else:
                        nc.scalar.activation(
                            out=qk_sb[:, m, ns * 512:(ns + 1) * 512],
                            in_=ps[:],
                            func=mybir.ActivationFunctionType.Identity,
                            bias=bqk_sb[:, m:m + 1],
                            scale=scale,
                        )

            def emit_v_aligned(a):
                # aligned v chunk a covers seq [128a, 128a+128); the
                # 64-shifted chunks are assembled later by partition-shift
                # DMAs instead of redundant matmuls
                off = 128 * a
                ps = psv.tile([128, 256], F32, tag="psv")
                for kt in range(8):
                    nc.tensor.matmul(
                        out=ps[:],
                        lhsT=xT_sb[:, kt, off:off + 128],
                        rhs=wv_sb[:, kt, :],
                        start=(kt == 0),
                        stop=(kt == 7),
                    )
                nc.scalar.copy(
                    out=v_sb[:, 16 + a],
                    in_=ps[:].rearrange("p (h d) -> p h d", h=4),
                )

            def emit_vshift(g0, g1):
                # shifted chunk c = aligned(c) rows 64:128 ++ aligned(c+1)
                # rows 0:64; two strided SBUF->SBUF DMAs build chunks
                # [g0, g1) from aligned slots on the idle mid-kernel queue
                nc.sync.dma_start(
                    out=v_sb[0:64, g0:g1],
                    in_=v_sb[64:128, 16 + g0:16 + g1],
                )
                nc.sync.dma_start(
                    out=v_sb[64:128, g0:g1],
                    in_=v_sb[0:64, 17 + g0:17 + g1],
                )

            def emit_scores_pair(c4, hp, pair):
                # scores + exp for both heads, one ii-pair (256 queries).
                # hh=0 lives in PE rows 0-63, hh=1 in rows 64-127: the
                # half-interleaved order lets consecutive matmuls execute
                # concurrently in disjoint row-groups. Both heads' exp
                # results land in one combined SBUF tile so the band mask
                # multiply is a single DVE op (mask broadcast over hh).
                if c4 == 0 and pair == 0:
                    mslice = masks_sb[:, :, 0:2]      # (first, interior)
                elif c4 == 3 and pair == 1:
                    mslice = masks_sb[:, :, 1:3]      # (interior, last)
                else:
                    mslice = masks_sb[:, :, 1:2]      # interior (iw-bcast)
                # one 2-bank tile for both heads: hh0 regions in bank A,
                # hh1 in bank B; a single exp ACTIVATE then covers the
                # whole unit (halves ACT per-op overhead)
                sct = pssc.tile([128, 2, 2, 2, 128], F32, tag="pssc",
                                name="sc")
                scr = sct[:].rearrange("p h a b c -> p (h a b) c")
                i0 = c4 * 4 + pair * 2
                if 1 <= i0 - 1 and i0 + 1 <= 14:
                    # interior pair: chunk i0 is shared by both query
                    # blocks (as cB of i0 and cA of i0+1) and its two
                    # score regions (iw0,half1)+(iw1,half0) are adjacent
                    # in the psum tile, so the pair needs only 3 matmuls
                    # per head: 128 + 256 + 128 columns
                    plan = [
                        (OFFS[i0 - 1], i0 * 128, 128, (0, 1)),
                        (OFFS[i0], i0 * 128, 256, (1, 3)),
                        (OFFS[i0 + 1], (i0 + 1) * 128, 128, (3, 4)),
                    ]
                else:
                    plan = []
                    for iw in range(2):
                        cA, cB = _chunk_pair(i0 + iw)
                        for half, cc in enumerate((cA, cB)):
                            plan.append((OFFS[cc], (i0 + iw) * 128, 128,
                                         (iw * 2 + half, iw * 2 + half + 1)))
                for mi, (koff, qoff, qn, (r0, r1)) in enumerate(plan):
                    for hh in range(2):
                        po = hh * 64
                        nc.tensor.matmul(
                            out=scr[:, 4 * hh + r0:4 * hh + r1, :],
                            lhsT=qk_sb[po:po + 64, 2 + hp,
                                       koff:koff + 128],
                            rhs=qk_sb[po:po + 64, hp, qoff:qoff + qn],
                            start=(mi == 0),
                            stop=(mi == len(plan) - 1),
                        )
                ex = epool.tile([128, 2, 2, 2, 128], F16, tag="exp",
                                name="ex")
                nc.scalar.activation(
                    out=ex[:],
                    in_=sct[:],
                    func=mybir.ActivationFunctionType.Exp,
                    bias=negc_sb[:],
                )
                nc.vector.tensor_mul(
                    out=ex[:],
                    in0=ex[:],
                    in1=mslice.broadcast_to([128, 2, 2, 2, 128]),
                )
                return ex

            def emit_av_norm_block(c4, hp, exs):
                # denominator-broadcast first: one ones-matmul per
                # (hh, pair, half) sums exp over keys straight from the ex
                # SBUF tile into bc rows 64*hh..64*hh+64 — the denominator
                # arrives already broadcast, so the reciprocal runs on DVE
                # concurrently with the AV matmuls.
                # each disjoint psum region's first write must carry
                # start=True: the has_written clear does not cover
                # partitions/regions the starting matmul doesn't write
                bc = ps512.tile([128, 512], F32, tag="ps512")
                for hh in range(2):
                    for pair in range(2):
                        for half in range(2):
                            nc.tensor.matmul(
                                out=bc[64 * hh:64 * hh + 64,
                                       256 * pair:256 * pair + 256],
                                lhsT=ones_sb[:],
                                rhs=exs[pair][:, hh, :, half, :],
                                start=(half == 0),
                                stop=(half == 1),
                            )
                bcs = wpool.tile([128, 512], F32, tag="bcs")
                nc.vector.reciprocal_approx_fast(out=bcs[:], in_=bc[:])

                # ii-outer so each 256-query half of the block finalizes
                # early; the per-pair aoT muls then run on DVE while the
                # second half's AV matmuls still stream on the PE
                avt = psav.tile([128, 4, 128], F32, tag="psav")
                for ii in range(4):
                    pair, iw = divmod(ii, 2)
                    cA, cB = _chunk_pair(c4 * 4 + ii)
                    for half, cc in enumerate((cA, cB)):
                        for hh in range(2):
                            nc.tensor.matmul(
                                out=avt[64 * hh:64 * hh + 64, ii, :],
                                lhsT=v_sb[:, VSLOT[cc], 2 * hp + hh, :],
                                rhs=exs[pair][:, hh, iw, half, :],
                                start=(half == 0),
                                stop=(half == 1),
                            )
                for pair in range(2):
                    sl = slice(c4 * 512 + 256 * pair,
                               c4 * 512 + 256 * pair + 256)
                    csl = slice(2 * pair, 2 * pair + 2)
                    nc.vector.tensor_mul(
                        out=aoT_sb[:, hp, sl],
                        in0=avt[:, csl, :].rearrange("p a b -> p (a b)"),
                        in1=bcs[:, 256 * pair:256 * pair + 256],
                    )

            yslot = {}

            def emit_outproj_st(st):
                # output staged in 2-st slots: one 512 KB DMA per slot with
                # 2 KB-contiguous DRAM rows (vs per-(st,nn) 128 KB DMAs
                # whose 1 KB rows ran the output path at ~100 GB/s and
                # backpressured ACT through the ysb pool)
                if st >= 14:
                    g, sub = st, 0  # final tiles ship solo to trim the tail
                else:
                    g, sub = divmod(st, 2)
                if sub == 0:
                    nst = 1 if st >= 14 else 2
                    yslot[g] = ypool.tile([128, nst, 1024], F16, tag="ysb",
                                          name="ysb")
                for nn in range(2):
                    # after the last scores unit the pssc banks are free:
                    # alternate the tail's outproj PSUM between pssc and
                    # ps512 so the final o-units stop stalling on the
                    # 2-deep ps512 ring
                    if st >= 12 and nn == 0:
                        ps = pssc.tile([128, 512], F32, tag="pssc",
                                       name="otail")
                    else:
                        ps = ps512.tile([128, 512], F32, tag="ps512")
                    for hp2 in range(2):
                        nc.tensor.matmul(
                            out=ps[:],
                            lhsT=aoT_sb[:, hp2, st * 128:(st + 1) * 128],
                            rhs=wout_sb[:, hp2, nn * 512:(nn + 1) * 512],
                            start=(hp2 == 0),
                            stop=(hp2 == 1),
                        )
                    # alternate the PSUM->SBUF copies over ACT/DVE so the
                    # two copies of one st run concurrently instead of
                    # serializing on ACT (which also owns the exps)
                    if nn == 0:
                        nc.scalar.copy(
                            out=yslot[g][:, sub, nn * 512:(nn + 1) * 512],
                            in_=ps[:],
                        )
                    else:
                        nc.vector.tensor_copy(
                            out=yslot[g][:, sub, nn * 512:(nn + 1) * 512],
                            in_=ps[:],
                        )
                if sub == 1 or st >= 14:
                    r0 = g * 256 if st < 14 else st * 128
                    r1 = r0 + (256 if st < 14 else 128)
                    nc.sync.dma_start(
                        out=out_d[r0:r1, :].rearrange(
                            "(s p) n -> p s n", p=128),
                        in_=yslot.pop(g)[:],
                    )

            # ---- emission schedule: per-pair scores -> filler -> AV so
            # the PE has independent work while ACT runs exp; projections
            # and outproj tiles are the fillers ----
            qk0_tiles = emit_qk_ns0_ktouter()

            # Schedule: tokens are ("s", c4, hp, pair) scores units,
            # ("a", c4, hp) per-block AV+norm, and fillers ("v", c) /
            # ("o", st) / ("qk", ns). Scores of pair-0 units for c4=0 only
            # need qk-ns0, so they run before QK1 (whose xT lands later).
            # va0 fills the PE while qk0's (m0, m2) bias ops run; the
            # (m1, m3) ops land after the first scores unit so it does not
            # falsely wait on them.
            SEQ = [
                ("va", 0),
                ("s", 0, 0, 0),
                ("qk0b",),
                ("qk", 1),
                ("s", 0, 1, 0), ("va", 1), ("va", 2), ("va", 3),
                ("s", 0, 0, 1), ("va", 4), ("sh", 0, 4), ("va", 5),
                ("a", 0, 0),
                ("s", 0, 1, 1), ("va", 6), ("a", 0, 1),
                ("qk", 2),
                ("s", 1, 0, 0), ("va", 7), ("va", 8), ("sh", 4, 8),
                ("s", 1, 0, 1), ("o", 0), ("a", 1, 0),
                ("s", 1, 1, 0), ("o", 1),
                ("s", 1, 1, 1), ("va", 9), ("a", 1, 1),
                ("qk", 3),
                ("s", 2, 0, 0), ("va", 10), ("va", 11),
                ("s", 2, 0, 1), ("va", 12), ("sh", 8, 12), ("o", 2),
                ("a", 2, 0),
                ("s", 2, 1, 0), ("o", 3), ("o", 4),
                ("s", 2, 1, 1), ("va", 13), ("o", 5), ("a", 2, 1),
                ("va", 14), ("va", 15), ("sh", 12, 15),
                ("s", 3, 0, 0), ("o", 6), ("o", 7),
                ("s", 3, 0, 1), ("o", 8), ("a", 3, 0),
                ("s", 3, 1, 0), ("o", 9), ("o", 10),
                ("s", 3, 1, 1), ("o", 11), ("a", 3, 1),
                ("o", 12), ("o", 13), ("o", 14), ("o", 15),
            ]

            exmap = {}
            for tok in SEQ:
                if tok[0] == "s":
                    _, c4, hp, pair = tok
                    exmap[(c4, hp, pair)] = emit_scores_pair(c4, hp, pair)
                elif tok[0] == "a":
                    _, c4, hp = tok
                    emit_av_norm_block(
                        c4, hp,
                        (exmap.pop((c4, hp, 0)), exmap.pop((c4, hp, 1))),
                    )
                elif tok[0] == "qk0b":
                    _qk0_ts(qk0_tiles, (1, 3))
                elif tok[0] == "va":
                    emit_v_aligned(tok[1])
                elif tok[0] == "sh":
                    emit_vshift(tok[1], tok[2])
                elif tok[0] == "o":
                    emit_outproj_st(tok[1])
                else:
                    emit_qk_ns(tok[1])

    nc.compile()
    return nc


_NC = None


def _get_program():
    global _NC
    if _NC is None:
        _NC = _build_program()
    return _NC


def _make_in_maps(x, w_qkv, b_qkv, w_out):
    masks = _build_masks()

    in_maps = []
    for c in range(8):
        b, hg = divmod(c, 4)
        cq = 256 * hg
        wqk = np.concatenate(
            [w_qkv[:, cq:cq + 256], w_qkv[:, 1024 + cq:1024 + cq + 256]],
            axis=1,
        ).astype(np.float16)
        bqk = np.empty((128, 4), np.float32)
        bqk[:, 0] = b_qkv[cq:cq + 128] * SCALE
        bqk[:, 1] = b_qkv[cq + 128:cq + 256] * SCALE
        bqk[:, 2] = b_qkv[1024 + cq:1024 + cq + 128]
        bqk[:, 3] = b_qkv[1024 + cq + 128:1024 + cq + 256]
        in_maps.append({
            "xT": np.ascontiguousarray(x[b].T).astype(np.float16),
            "wqk": wqk,
            "wv": w_qkv[:, 2048 + cq:2048 + cq + 256].astype(np.float16),
            "wout": w_out[cq:cq + 256, :].astype(np.float16),
            "bqk": bqk,
            "masks": masks,
        })
    return in_maps


def kernel(x, w_qkv, b_qkv, w_out, b_out):
    x = np.asarray(x, np.float32)
    w_qkv = np.asarray(w_qkv, np.float32)
    b_qkv = np.asarray(b_qkv, np.float32)
    w_out = np.asarray(w_out, np.float32)
    b_out = np.asarray(b_out, np.float32)

    in_maps = _make_in_maps(x, w_qkv, b_qkv, w_out)
    nc = _get_program()
    res = run_bass_kernel_spmd(nc, in_maps, list(range(8)))

    b_v = b_qkv[2048:]
    bias_all = b_out + b_v @ w_out  # folds the (untracked) v-bias
    y = np.empty((B, S, D), np.float32)
    for b in range(B):
        acc = np.zeros((S, D), np.float32)
        for hg in range(4):
            acc += res.results[4 * b + hg]["out"].astype(np.float32)
        y[b] = acc + bias_all
    return y

